# revision 17
# baseline (speedup 1.0000x reference)
"""Trainium2 Bass kernel for nn_BasicBlock (rulebook sparse conv x2 + BN + ReLU + residual).

Strategy (8 NeuronCores, data-parallel over the N=200000 active voxels):
  - each core owns a contiguous shard of 25000 voxels (padded to 49 tiles of 512)
  - per (tile, k, j): single-index indirect DMA gathers 128 neighbor rows
    (bf16, 256B each) from the table (x for conv1, all-gathered h for conv2);
    masked entries point at an out-of-bounds index and are skipped by the DMA
    bounds check (tiles pre-zeroed, so skipped rows contribute exact zeros)
  - gathered [voxel, ch] tiles are transposed on the PE (matmul vs identity),
    evacuated PSUM->SBUF to bf16, then W_k-stationary bf16 matmuls accumulate
    out^T [Cout, 512] in PSUM over the 27 offsets
  - BN stats (sum / sum-of-squares) reduce along the free axis of out^T and
    are all-reduced across cores; BN+ReLU applied in the transposed domain
    (per-partition scale/bias); h transposed back and all-gathered (bf16)
  - final: BN2 + identity residual (x) + ReLU, output fp32
"""
import sys, os, types, contextlib

sys.path.insert(0, '/opt/trn_rl_repo')
sys.path.insert(0, '/root/.axon_site')

import numpy as np

FULL_CFG = dict(
    n_cores=8,
    n_rows=200000,   # table rows (N)
    shard=25000,     # voxels per core
    nt=49,           # 512-voxel tiles per core (pads shard to 25088)
    k=27,
    c=128,
)

BIG = 1 << 21


def _install_trace_hook():
    """Register the NTFF profile hook (missing antenv.axon_hooks in this image)."""
    try:
        import antenv
        if "antenv.axon_hooks" not in sys.modules:
            mod = types.ModuleType("antenv.axon_hooks")
            mod._hook = None
            mod.set_axon_ntff_profile_hook = lambda h: setattr(mod, "_hook", h)
            mod.get_axon_ntff_profile_hook = lambda: mod._hook
            sys.modules["antenv.axon_hooks"] = mod
            antenv.axon_hooks = mod
            from trn_agent_boot.trn_boot import _ntff_profile_via_ctypes
            hook = _ntff_profile_via_ctypes('/opt/axon/libaxon_pjrt.so')
            if hook is not None:
                mod.set_axon_ntff_profile_hook(hook)
    except Exception:
        pass


def build_nc(cfg):
    import concourse.bass as bass
    import concourse.bacc as bacc
    import concourse.tile as tile
    from concourse import mybir
    from concourse.masks import make_identity

    P = 128
    C = cfg["c"]
    K = cfg["k"]
    NT = cfg["nt"]
    SHARD = cfg["shard"]
    NROWS = cfg["n_rows"]
    NCORES = cfg["n_cores"]
    NPAD = NT * 512          # 25088 padded voxels per core
    GROWS = NCORES * NPAD    # padded global h table rows
    M = K * 4                # index slots per partition per tile
    f32 = mybir.dt.float32
    bf16 = mybir.dt.bfloat16
    i32 = mybir.dt.int32
    AF = mybir.ActivationFunctionType
    ALU = mybir.AluOpType
    AX = mybir.AxisListType

    nc = bacc.Bacc("TRN2", target_bir_lowering=False)
    xtab = nc.dram_tensor("xtab", [NROWS + 1, C], bf16, kind="ExternalInput")
    xres = nc.dram_tensor("xres", [SHARD, C], f32, kind="ExternalInput")
    idx1_d = nc.dram_tensor("idx1", [NT, P, M], i32, kind="ExternalInput")
    idx2_d = nc.dram_tensor("idx2", [NT, P, M], i32, kind="ExternalInput")
    W1_in = nc.dram_tensor("W1h", [P, K * C], bf16, kind="ExternalInput")
    W2_in = nc.dram_tensor("W2h", [P, K * C], bf16, kind="ExternalInput")
    gam1 = nc.dram_tensor("gamma1", [C], f32, kind="ExternalInput")
    bet1 = nc.dram_tensor("beta1", [C], f32, kind="ExternalInput")
    gam2 = nc.dram_tensor("gamma2", [C], f32, kind="ExternalInput")
    bet2 = nc.dram_tensor("beta2", [C], f32, kind="ExternalInput")
    out_d = nc.dram_tensor("out", [SHARD, C], f32, kind="ExternalOutput")

    rgroups = [list(range(NCORES))]
    inv_n = 1.0 / (SHARD * NCORES)

    with tile.TileContext(nc) as tc:
        with contextlib.ExitStack() as ctx:
            # DRAM bounce pools (tracked by Tile so collectives order correctly)
            hgat_pool = ctx.enter_context(tc.tile_pool(name="hgat", bufs=1, space="DRAM"))
            dram_pool = ctx.enter_context(tc.tile_pool(name="drb", bufs=1, space="DRAM"))
            h_gat = hgat_pool.tile([GROWS + 1, C], bf16)       # gather table for conv2 (+ zero row)
            h_shard = dram_pool.tile([NPAD, C], bf16)
            st_in = [dram_pool.tile([P, 2], f32, name=f"st_in{i}") for i in range(2)]
            st_out = [dram_pool.tile([P, 2], f32, name=f"st_out{i}") for i in range(2)]

            perm = ctx.enter_context(tc.tile_pool(name="perm", bufs=1))
            ipool = ctx.enter_context(tc.tile_pool(name="ip", bufs=3))
            gpool = ctx.enter_context(tc.tile_pool(name="g", bufs=24))
            rpool = ctx.enter_context(tc.tile_pool(name="r", bufs=6))
            spool = ctx.enter_context(tc.tile_pool(name="s", bufs=4))
            ppool = ctx.enter_context(tc.tile_pool(name="ps", bufs=3, space="PSUM"))
            opool = ctx.enter_context(tc.tile_pool(name="po", bufs=2, space="PSUM"))

            W1sb = perm.tile([P, K * C], bf16)
            W2sb = perm.tile([P, K * C], bf16)
            id32 = perm.tile([P, P], f32)
            id16 = perm.tile([P, P], bf16)
            zg16 = perm.tile([P, C], bf16)
            hT = perm.tile([P, NPAD], bf16)         # conv1 out^T; reused as conv2 out^T
            s1t = [perm.tile([P, NT], f32, name=f"s1t{i}") for i in range(2)]
            s2t = [perm.tile([P, NT], f32, name=f"s2t{i}") for i in range(2)]
            gb = {n: perm.tile([P, 1], f32, name=f"gb_{n}") for n in ("g1", "b1", "g2", "b2")}
            ab = {n: perm.tile([P, 1], f32, name=f"ab_{n}") for n in ("a1", "bb1", "a2", "bb2")}
            sc = {n: perm.tile([P, 1], f32, name=f"sc_{n}") for n in ("mu", "ex2", "var", "rsig", "tmp")}
            stpack = [perm.tile([P, 2], f32, name=f"stpack{i}") for i in range(2)]
            stred = [perm.tile([P, 2], f32, name=f"stred{i}") for i in range(2)]

            make_identity(nc, id32[:])
            nc.vector.tensor_copy(id16[:], id32[:])
            nc.gpsimd.memset(zg16[:], 0.0)
            # zero row at the end of the conv2 gather table (masked entries hit it)
            nc.sync.dma_start(out=h_gat[GROWS:GROWS + 1, :], in_=zg16[:1, :])

            nc.sync.dma_start(W1sb[:], W1_in[:])
            nc.sync.dma_start(W2sb[:], W2_in[:])
            nc.sync.dma_start(gb["g1"][:], gam1[:, None])
            nc.sync.dma_start(gb["b1"][:], bet1[:, None])
            nc.sync.dma_start(gb["g2"][:], gam2[:, None])
            nc.sync.dma_start(gb["b2"][:], bet2[:, None])

            def conv(table_ap, bound, idx_d, Wsb, s1, s2, epilogue):
                for t in range(NT):
                    it = ipool.tile([P, M], i32, tag="it")
                    nc.sync.dma_start(it[:], idx_d[t])
                    po = opool.tile([P, 512], f32, space="PSUM", tag="po")
                    for kk in range(K):
                        pt = ppool.tile([P, 512], f32, space="PSUM", tag="pt")
                        for j in range(4):
                            m = kk * 4 + j
                            gt = gpool.tile([P, C], bf16, tag="gt")
                            nc.gpsimd.indirect_dma_start(
                                out=gt[:], out_offset=None, in_=table_ap,
                                in_offset=bass.IndirectOffsetOnAxis(
                                    ap=it[:, m:m + 1], axis=0),
                                bounds_check=bound, oob_is_err=False)
                            nc.tensor.matmul(pt[:, j * P:(j + 1) * P],
                                             lhsT=gt[:], rhs=id16[:],
                                             start=(j == 0), stop=(j == 3))
                        rhs = rpool.tile([P, 512], bf16, tag="rhs")
                        nc.scalar.copy(rhs[:], pt[:])
                        nc.tensor.matmul(po[:], lhsT=Wsb[:, kk * C:(kk + 1) * C], rhs=rhs[:],
                                         start=(kk == 0), stop=(kk == K - 1))
                    nc.vector.reduce_sum(s1[:, t:t + 1], po[:], axis=AX.X)
                    sq = spool.tile([P, 512], f32, tag="sq")
                    nc.scalar.activation(sq[:], po[:], AF.Square, accum_out=s2[:, t:t + 1])
                    epilogue(t, po)

            def stats_allreduce(s1, s2, i, gamma, beta, a_t, b_t):
                # reduce over tile columns, pack, all-reduce, compute a=gamma*rsig, b=beta-mu*a
                nc.vector.reduce_sum(stpack[i][:, 0:1], s1[:], axis=AX.X)
                nc.vector.reduce_sum(stpack[i][:, 1:2], s2[:], axis=AX.X)
                nc.sync.dma_start(st_in[i][:], stpack[i][:])
                nc.gpsimd.collective_compute(
                    "AllReduce", ALU.add, replica_groups=rgroups,
                    ins=[st_in[i][:]], outs=[st_out[i][:]])
                nc.sync.dma_start(stred[i][:], st_out[i][:])
                nc.vector.tensor_scalar_mul(sc["mu"][:], stred[i][:, 0:1], inv_n)
                nc.vector.tensor_scalar_mul(sc["ex2"][:], stred[i][:, 1:2], inv_n)
                nc.vector.tensor_tensor(out=sc["var"][:], in0=sc["mu"][:], in1=sc["mu"][:], op=ALU.mult)
                nc.vector.tensor_tensor(out=sc["var"][:], in0=sc["ex2"][:], in1=sc["var"][:], op=ALU.subtract)
                nc.vector.tensor_scalar_add(sc["var"][:], sc["var"][:], 1e-5)
                nc.scalar.activation(sc["tmp"][:], sc["var"][:], AF.Sqrt)
                nc.vector.reciprocal(sc["rsig"][:], sc["tmp"][:])
                nc.vector.tensor_tensor(out=a_t[:], in0=gamma[:], in1=sc["rsig"][:], op=ALU.mult)
                nc.vector.tensor_tensor(out=sc["tmp"][:], in0=sc["mu"][:], in1=a_t[:], op=ALU.mult)
                nc.vector.tensor_tensor(out=b_t[:], in0=beta[:], in1=sc["tmp"][:], op=ALU.subtract)

            # ================= conv1 =================
            def ep1(t, po):
                nc.vector.tensor_copy(hT[:, t * 512:(t + 1) * 512], po[:])

            conv(xtab[:], NROWS, idx1_d, W1sb, s1t[0], s2t[0], ep1)
            stats_allreduce(s1t[0], s2t[0], 0, gb["g1"], gb["b1"], ab["a1"], ab["bb1"])

            # BN1 + ReLU in ^T domain, transpose back, write row-major bf16 shard;
            # all-gather h in 7-tile chunks so the collective overlaps the
            # transpose-back of later chunks (h_gat layout is chunk-major:
            # global row (q, r) lives at ((r//3584)*8 + q)*3584 + r%3584)
            CH = 7 * 512
            for c7 in range(7):
                for t in range(c7 * 7, (c7 + 1) * 7):
                    cs = slice(t * 512, (t + 1) * 512)
                    nc.scalar.activation(hT[:, cs], hT[:, cs], AF.Relu,
                                         bias=ab["bb1"][:], scale=ab["a1"][:])
                    ptb = ppool.tile([P, 512], f32, space="PSUM", tag="pt")
                    for j in range(4):
                        nc.tensor.matmul(ptb[:, j * P:(j + 1) * P],
                                         lhsT=hT[:, t * 512 + j * P: t * 512 + (j + 1) * P],
                                         rhs=id16[:], start=(j == 0), stop=(j == 3))
                    hsb = rpool.tile([P, 512], bf16, tag="rhs")
                    nc.vector.tensor_copy(hsb[:], ptb[:])
                    nc.sync.dma_start(
                        out=h_shard[t * 512:(t + 1) * 512, :].rearrange("(j p) c -> p j c", j=4),
                        in_=hsb[:].rearrange("p (j c) -> p j c", j=4))
                nc.gpsimd.collective_compute(
                    "AllGather", ALU.bypass, replica_groups=rgroups,
                    ins=[h_shard[c7 * CH:(c7 + 1) * CH, :]],
                    outs=[h_gat[c7 * NCORES * CH:(c7 + 1) * NCORES * CH, :]])

            # ================= conv2 =================
            oT = hT  # reuse conv1 buffer for conv2 out^T

            def ep2(t, po):
                nc.vector.tensor_copy(oT[:, t * 512:(t + 1) * 512], po[:])

            conv(h_gat[:], GROWS, idx2_d, W2sb, s1t[1], s2t[1], ep2)
            stats_allreduce(s1t[1], s2t[1], 1, gb["g2"], gb["b2"], ab["a2"], ab["bb2"])

            # final: BN2 (^T domain) -> transpose back -> + x -> ReLU -> out
            for t in range(NT):
                cs = slice(t * 512, (t + 1) * 512)
                tmp = rpool.tile([P, 512], bf16, tag="rhs")
                nc.vector.tensor_scalar(out=tmp[:], in0=oT[:, cs], scalar1=ab["a2"][:],
                                        scalar2=ab["bb2"][:], op0=ALU.mult, op1=ALU.add)
                pf = ppool.tile([P, 512], f32, space="PSUM", tag="pt")
                for j in range(4):
                    nc.tensor.matmul(pf[:, j * P:(j + 1) * P],
                                     lhsT=tmp[:, j * P:(j + 1) * P],
                                     rhs=id16[:], start=(j == 0), stop=(j == 3))
                xt = spool.tile([P, 512], f32, tag="xt")
                res = spool.tile([P, 512], f32, tag="res")
                for j in range(4):
                    r0 = t * 512 + j * P
                    rj = min(P, SHARD - r0)
                    if rj <= 0:
                        break
                    nc.sync.dma_start(out=xt[:rj, j * P:(j + 1) * P],
                                      in_=xres[r0:r0 + rj, :])
                nc.vector.tensor_tensor(out=res[:], in0=pf[:], in1=xt[:], op=ALU.add)
                ro = spool.tile([P, 512], f32, tag="ro")
                nc.scalar.activation(ro[:], res[:], AF.Relu)
                for j in range(4):
                    r0 = t * 512 + j * P
                    rj = min(P, SHARD - r0)
                    if rj <= 0:
                        break
                    nc.sync.dma_start(out=out_d[r0:r0 + rj, :], in_=ro[:rj, j * P:(j + 1) * P])

    nc.compile()
    return nc


def prepare_in_maps(cfg, x, W1, gamma1, beta1, W2, gamma2, beta2, neighbor_idx, neighbor_mask):
    import ml_dtypes
    bf16 = ml_dtypes.bfloat16
    P = 128
    K = cfg["k"]
    NT = cfg["nt"]
    SHARD = cfg["shard"]
    NCORES = cfg["n_cores"]
    NPAD = NT * 512

    idx32 = np.asarray(neighbor_idx, dtype=np.int64).astype(np.int32)
    mask32 = np.asarray(neighbor_mask, dtype=np.int32)
    x = np.ascontiguousarray(np.asarray(x, dtype=np.float32))
    xtab = np.zeros((x.shape[0] + 1, P), bf16)   # + zero row for masked entries
    xtab[:-1] = x.astype(bf16)
    W1h = np.ascontiguousarray(
        np.asarray(W1, np.float32).transpose(1, 0, 2).reshape(P, K * P).astype(bf16))
    W2h = np.ascontiguousarray(
        np.asarray(W2, np.float32).transpose(1, 0, 2).reshape(P, K * P).astype(bf16))

    # conv2 table position of global row g (h table is chunk-major:
    # ((r//3584)*8 + q)*3584 + r%3584)
    q, r = idx32 // SHARD, idx32 % SHARD
    idx_h = ((r // 3584) * NCORES + q) * 3584 + r % 3584

    vv = np.arange(NPAD).reshape(NT, 4, P)       # local voxel id = 512t + 128j + p
    valid = vv < SHARD
    in_maps = []
    for c in range(NCORES):
        gid = c * SHARD + np.where(valid, vv, 0)
        mb = np.where(valid[..., None], mask32[gid], 0)      # [NT, 4, P, K]
        keep = mb.astype(bool)
        i1 = np.where(keep, idx32[gid], 200000)        # masked -> zero row
        i2 = np.where(keep, idx_h[gid], NCORES * NPAD)
        # layout [NT, P, K*4]: slot m = k*4 + j for voxel 512t + 128j + p
        def lay(a, dt):
            return np.ascontiguousarray(a.transpose(0, 2, 3, 1).reshape(NT, P, K * 4).astype(dt))
        in_maps.append({
            "xtab": xtab, "xres": np.ascontiguousarray(x[c * SHARD:(c + 1) * SHARD]),
            "idx1": lay(i1, np.int32), "idx2": lay(i2, np.int32),
            "W1h": W1h, "W2h": W2h,
            "gamma1": np.asarray(gamma1, np.float32), "beta1": np.asarray(beta1, np.float32),
            "gamma2": np.asarray(gamma2, np.float32), "beta2": np.asarray(beta2, np.float32),
        })
    return in_maps


_NC_CACHE = {}


def kernel(**inputs):
    _install_trace_hook()
    from concourse import bass_utils

    cfg = FULL_CFG
    key = "full"
    if key not in _NC_CACHE:
        _NC_CACHE[key] = build_nc(cfg)
    nc = _NC_CACHE[key]
    in_maps = prepare_in_maps(cfg, **inputs)
    trace = bool(int(os.environ.get("BASS_KERNEL_TRACE", "0")))
    res = bass_utils.run_bass_kernel_spmd(
        nc, in_maps, core_ids=list(range(cfg["n_cores"])), trace=trace)
    out = np.concatenate([res.results[c]["out"] for c in range(cfg["n_cores"])], axis=0)
    if trace:
        kernel.last_exec_time_ns = res.exec_time_ns
    return out


# revision 20
# speedup vs baseline: 1.2319x; 1.2319x over previous
"""Trainium2 Bass kernel for nn_BasicBlock (rulebook sparse conv x2 + BN + ReLU + residual).

Strategy (8 NeuronCores, data-parallel over the N=200000 active voxels):
  - each core owns a contiguous shard of 25000 voxels (padded to 49 tiles of 512)
  - per (tile, k, j): single-index indirect DMA gathers 128 neighbor rows
    (bf16, 256B each) from the table (x for conv1, all-gathered h for conv2);
    masked entries point at an out-of-bounds index and are skipped by the DMA
    bounds check (tiles pre-zeroed, so skipped rows contribute exact zeros)
  - gathered [voxel, ch] tiles are transposed on the PE (matmul vs identity),
    evacuated PSUM->SBUF to bf16, then W_k-stationary bf16 matmuls accumulate
    out^T [Cout, 512] in PSUM over the 27 offsets
  - BN stats (sum / sum-of-squares) reduce along the free axis of out^T and
    are all-reduced across cores; BN+ReLU applied in the transposed domain
    (per-partition scale/bias); h transposed back and all-gathered (bf16)
  - final: BN2 + identity residual (x) + ReLU, output fp32
"""
import sys, os, types, contextlib

sys.path.insert(0, '/opt/trn_rl_repo')
sys.path.insert(0, '/root/.axon_site')

import numpy as np

FULL_CFG = dict(
    n_cores=8,
    n_rows=200000,   # table rows (N)
    shard=25000,     # voxels per core
    nt=49,           # 512-voxel tiles per core (pads shard to 25088)
    k=27,
    c=128,
)

BIG = 1 << 21


def _install_trace_hook():
    """Register the NTFF profile hook (missing antenv.axon_hooks in this image)."""
    try:
        import antenv
        if "antenv.axon_hooks" not in sys.modules:
            mod = types.ModuleType("antenv.axon_hooks")
            mod._hook = None
            mod.set_axon_ntff_profile_hook = lambda h: setattr(mod, "_hook", h)
            mod.get_axon_ntff_profile_hook = lambda: mod._hook
            sys.modules["antenv.axon_hooks"] = mod
            antenv.axon_hooks = mod
            from trn_agent_boot.trn_boot import _ntff_profile_via_ctypes
            hook = _ntff_profile_via_ctypes('/opt/axon/libaxon_pjrt.so')
            if hook is not None:
                mod.set_axon_ntff_profile_hook(hook)
    except Exception:
        pass


def build_nc(cfg):
    import concourse.bass as bass
    import concourse.bacc as bacc
    import concourse.tile as tile
    from concourse import mybir
    from concourse.masks import make_identity

    P = 128
    C = cfg["c"]
    K = cfg["k"]
    NT = cfg["nt"]
    SHARD = cfg["shard"]
    NROWS = cfg["n_rows"]
    NCORES = cfg["n_cores"]
    NPAD = NT * 512          # 25088 padded voxels per core
    GROWS = NCORES * NPAD    # padded global h table rows
    M = K * 4                # index slots per partition per tile
    f32 = mybir.dt.float32
    bf16 = mybir.dt.bfloat16
    i32 = mybir.dt.int32
    AF = mybir.ActivationFunctionType
    ALU = mybir.AluOpType
    AX = mybir.AxisListType

    nc = bacc.Bacc("TRN2", target_bir_lowering=False)
    xtab = nc.dram_tensor("xtab", [NROWS, C], bf16, kind="ExternalInput")
    xres = nc.dram_tensor("xres", [SHARD, C], f32, kind="ExternalInput")
    idx1_d = nc.dram_tensor("idx1", [NT, P, M], i32, kind="ExternalInput")
    idx2_d = nc.dram_tensor("idx2", [NT, P, M], i32, kind="ExternalInput")
    W1_in = nc.dram_tensor("W1h", [P, K * C], bf16, kind="ExternalInput")
    W2_in = nc.dram_tensor("W2h", [P, K * C], bf16, kind="ExternalInput")
    gam1 = nc.dram_tensor("gamma1", [C], f32, kind="ExternalInput")
    bet1 = nc.dram_tensor("beta1", [C], f32, kind="ExternalInput")
    gam2 = nc.dram_tensor("gamma2", [C], f32, kind="ExternalInput")
    bet2 = nc.dram_tensor("beta2", [C], f32, kind="ExternalInput")
    out_d = nc.dram_tensor("out", [SHARD, C], f32, kind="ExternalOutput")

    rgroups = [list(range(NCORES))]
    inv_n = 1.0 / (SHARD * NCORES)

    with tile.TileContext(nc) as tc:
        with contextlib.ExitStack() as ctx:
            # DRAM bounce pools (tracked by Tile so collectives order correctly)
            hgat_pool = ctx.enter_context(tc.tile_pool(name="hgat", bufs=1, space="DRAM"))
            dram_pool = ctx.enter_context(tc.tile_pool(name="drb", bufs=1, space="DRAM"))
            h_gat = hgat_pool.tile([GROWS, C], bf16)           # gather table for conv2
            h_shard = dram_pool.tile([NPAD, C], bf16)
            st_in = [dram_pool.tile([P, 2], f32, name=f"st_in{i}") for i in range(2)]
            st_out = [dram_pool.tile([P, 2], f32, name=f"st_out{i}") for i in range(2)]

            perm = ctx.enter_context(tc.tile_pool(name="perm", bufs=1))
            ipool = ctx.enter_context(tc.tile_pool(name="ip", bufs=3))
            gpool = ctx.enter_context(tc.tile_pool(name="g", bufs=24))
            rpool = ctx.enter_context(tc.tile_pool(name="r", bufs=6))
            spool = ctx.enter_context(tc.tile_pool(name="s", bufs=4))
            xpool = ctx.enter_context(tc.tile_pool(name="x", bufs=12))
            ppool = ctx.enter_context(tc.tile_pool(name="ps", bufs=3, space="PSUM"))
            opool = ctx.enter_context(tc.tile_pool(name="po", bufs=2, space="PSUM"))

            W1sb = perm.tile([P, K * C], bf16)
            W2sb = perm.tile([P, K * C], bf16)
            id32 = perm.tile([P, P], f32)
            id16 = perm.tile([P, P], bf16)
            zg16 = perm.tile([P, C], bf16)
            hT = perm.tile([P, NPAD], bf16)         # conv1 out^T; reused as conv2 out^T
            s1t = [perm.tile([P, NT], f32, name=f"s1t{i}") for i in range(2)]
            s2t = [perm.tile([P, NT], f32, name=f"s2t{i}") for i in range(2)]
            gb = {n: perm.tile([P, 1], f32, name=f"gb_{n}") for n in ("g1", "b1", "g2", "b2")}
            ab = {n: perm.tile([P, 1], f32, name=f"ab_{n}") for n in ("a1", "bb1", "a2", "bb2")}
            sc = {n: perm.tile([P, 1], f32, name=f"sc_{n}") for n in ("mu", "ex2", "var", "rsig", "tmp")}
            stpack = [perm.tile([P, 2], f32, name=f"stpack{i}") for i in range(2)]
            stred = [perm.tile([P, 2], f32, name=f"stred{i}") for i in range(2)]

            make_identity(nc, id32[:])
            nc.vector.tensor_copy(id16[:], id32[:])
            nc.gpsimd.memset(zg16[:], 0.0)

            nc.sync.dma_start(W1sb[:], W1_in[:])
            nc.sync.dma_start(W2sb[:], W2_in[:])
            nc.sync.dma_start(gb["g1"][:], gam1[:, None])
            nc.sync.dma_start(gb["b1"][:], bet1[:, None])
            nc.sync.dma_start(gb["g2"][:], gam2[:, None])
            nc.sync.dma_start(gb["b2"][:], bet2[:, None])

            def conv(table_ap, bound, idx_d, Wsb, s1, s2, epilogue):
                for t in range(NT):
                    it = ipool.tile([P, M], i32, tag="it")
                    nc.sync.dma_start(it[:], idx_d[t])
                    po = opool.tile([P, 512], f32, space="PSUM", tag="po")
                    for kk in range(K):
                        pt = ppool.tile([P, 512], f32, space="PSUM", tag="pt")
                        for j in range(4):
                            m = kk * 4 + j
                            gt = gpool.tile([P, C], bf16, tag="gt")
                            nc.vector.tensor_copy(gt[:], zg16[:])
                            nc.gpsimd.indirect_dma_start(
                                out=gt[:], out_offset=None, in_=table_ap,
                                in_offset=bass.IndirectOffsetOnAxis(
                                    ap=it[:, m:m + 1], axis=0),
                                bounds_check=bound, oob_is_err=False)
                            nc.tensor.matmul(pt[:, j * P:(j + 1) * P],
                                             lhsT=gt[:], rhs=id16[:],
                                             start=(j == 0), stop=(j == 3))
                        rhs = rpool.tile([P, 512], bf16, tag="rhs")
                        nc.scalar.copy(rhs[:], pt[:])
                        nc.tensor.matmul(po[:], lhsT=Wsb[:, kk * C:(kk + 1) * C], rhs=rhs[:],
                                         start=(kk == 0), stop=(kk == K - 1))
                    nc.vector.reduce_sum(s1[:, t:t + 1], po[:], axis=AX.X)
                    sq = spool.tile([P, 512], f32, tag="sq")
                    nc.scalar.activation(sq[:], po[:], AF.Square, accum_out=s2[:, t:t + 1])
                    epilogue(t, po)

            def stats_allreduce(s1, s2, i, gamma, beta, a_t, b_t):
                # reduce over tile columns, pack, all-reduce, compute a=gamma*rsig, b=beta-mu*a
                nc.vector.reduce_sum(stpack[i][:, 0:1], s1[:], axis=AX.X)
                nc.vector.reduce_sum(stpack[i][:, 1:2], s2[:], axis=AX.X)
                nc.sync.dma_start(st_in[i][:], stpack[i][:])
                nc.gpsimd.collective_compute(
                    "AllReduce", ALU.add, replica_groups=rgroups,
                    ins=[st_in[i][:]], outs=[st_out[i][:]])
                nc.sync.dma_start(stred[i][:], st_out[i][:])
                nc.vector.tensor_scalar_mul(sc["mu"][:], stred[i][:, 0:1], inv_n)
                nc.vector.tensor_scalar_mul(sc["ex2"][:], stred[i][:, 1:2], inv_n)
                nc.vector.tensor_tensor(out=sc["var"][:], in0=sc["mu"][:], in1=sc["mu"][:], op=ALU.mult)
                nc.vector.tensor_tensor(out=sc["var"][:], in0=sc["ex2"][:], in1=sc["var"][:], op=ALU.subtract)
                nc.vector.tensor_scalar_add(sc["var"][:], sc["var"][:], 1e-5)
                nc.scalar.activation(sc["tmp"][:], sc["var"][:], AF.Sqrt)
                nc.vector.reciprocal(sc["rsig"][:], sc["tmp"][:])
                nc.vector.tensor_tensor(out=a_t[:], in0=gamma[:], in1=sc["rsig"][:], op=ALU.mult)
                nc.vector.tensor_tensor(out=sc["tmp"][:], in0=sc["mu"][:], in1=a_t[:], op=ALU.mult)
                nc.vector.tensor_tensor(out=b_t[:], in0=beta[:], in1=sc["tmp"][:], op=ALU.subtract)

            # ================= conv1 =================
            def ep1(t, po):
                nc.vector.tensor_copy(hT[:, t * 512:(t + 1) * 512], po[:])

            conv(xtab[:], NROWS - 1, idx1_d, W1sb, s1t[0], s2t[0], ep1)
            stats_allreduce(s1t[0], s2t[0], 0, gb["g1"], gb["b1"], ab["a1"], ab["bb1"])

            # BN1 + ReLU in ^T domain, transpose back, write row-major bf16 shard;
            # all-gather h in 7-tile chunks so the collective overlaps the
            # transpose-back of later chunks (h_gat layout is chunk-major:
            # global row (q, r) lives at ((r//3584)*8 + q)*3584 + r%3584)
            CH = 7 * 512
            for c7 in range(7):
                for t in range(c7 * 7, (c7 + 1) * 7):
                    cs = slice(t * 512, (t + 1) * 512)
                    nc.scalar.activation(hT[:, cs], hT[:, cs], AF.Relu,
                                         bias=ab["bb1"][:], scale=ab["a1"][:])
                    ptb = ppool.tile([P, 512], f32, space="PSUM", tag="pt")
                    for j in range(4):
                        nc.tensor.matmul(ptb[:, j * P:(j + 1) * P],
                                         lhsT=hT[:, t * 512 + j * P: t * 512 + (j + 1) * P],
                                         rhs=id16[:], start=(j == 0), stop=(j == 3))
                    hsb = rpool.tile([P, 512], bf16, tag="rhs")
                    nc.vector.tensor_copy(hsb[:], ptb[:])
                    nc.sync.dma_start(
                        out=h_shard[t * 512:(t + 1) * 512, :].rearrange("(j p) c -> p j c", j=4),
                        in_=hsb[:].rearrange("p (j c) -> p j c", j=4))
                nc.gpsimd.collective_compute(
                    "AllGather", ALU.bypass, replica_groups=rgroups,
                    ins=[h_shard[c7 * CH:(c7 + 1) * CH, :]],
                    outs=[h_gat[c7 * NCORES * CH:(c7 + 1) * NCORES * CH, :]])

            # ================= conv2 =================
            oT = hT  # reuse conv1 buffer for conv2 out^T

            def ep2(t, po):
                nc.vector.tensor_copy(oT[:, t * 512:(t + 1) * 512], po[:])

            conv(h_gat[:], GROWS - 1, idx2_d, W2sb, s1t[1], s2t[1], ep2)
            stats_allreduce(s1t[1], s2t[1], 1, gb["g2"], gb["b2"], ab["a2"], ab["bb2"])

            # final: BN2 (^T domain) -> transpose back -> + x -> ReLU -> out
            for t in range(NT):
                cs = slice(t * 512, (t + 1) * 512)
                tmp = rpool.tile([P, 512], bf16, tag="rhs")
                nc.vector.tensor_scalar(out=tmp[:], in0=oT[:, cs], scalar1=ab["a2"][:],
                                        scalar2=ab["bb2"][:], op0=ALU.mult, op1=ALU.add)
                pf = ppool.tile([P, 512], f32, space="PSUM", tag="pt")
                for j in range(4):
                    nc.tensor.matmul(pf[:, j * P:(j + 1) * P],
                                     lhsT=tmp[:, j * P:(j + 1) * P],
                                     rhs=id16[:], start=(j == 0), stop=(j == 3))
                xt = xpool.tile([P, 512], f32, tag="xt")
                res = spool.tile([P, 512], f32, tag="res")
                if t < NT - 1:
                    nc.sync.dma_start(
                        out=xt[:].rearrange("p (j c) -> p j c", j=4),
                        in_=xres[t * 512:(t + 1) * 512, :].rearrange("(j p) c -> p j c", j=4))
                else:
                    for j in range(4):
                        r0 = t * 512 + j * P
                        rj = min(P, SHARD - r0)
                        if rj <= 0:
                            break
                        nc.sync.dma_start(out=xt[:rj, j * P:(j + 1) * P],
                                          in_=xres[r0:r0 + rj, :])
                nc.vector.tensor_tensor(out=res[:], in0=pf[:], in1=xt[:], op=ALU.add)
                ro = spool.tile([P, 512], f32, tag="ro")
                nc.scalar.activation(ro[:], res[:], AF.Relu)
                if t < NT - 1:
                    nc.sync.dma_start(
                        out=out_d[t * 512:(t + 1) * 512, :].rearrange("(j p) c -> p j c", j=4),
                        in_=ro[:].rearrange("p (j c) -> p j c", j=4))
                else:
                    for j in range(4):
                        r0 = t * 512 + j * P
                        rj = min(P, SHARD - r0)
                        if rj <= 0:
                            break
                        nc.sync.dma_start(out=out_d[r0:r0 + rj, :], in_=ro[:rj, j * P:(j + 1) * P])

    nc.compile()
    return nc


def prepare_in_maps(cfg, x, W1, gamma1, beta1, W2, gamma2, beta2, neighbor_idx, neighbor_mask):
    import ml_dtypes
    bf16 = ml_dtypes.bfloat16
    P = 128
    K = cfg["k"]
    NT = cfg["nt"]
    SHARD = cfg["shard"]
    NCORES = cfg["n_cores"]
    NPAD = NT * 512

    idx32 = np.asarray(neighbor_idx, dtype=np.int64).astype(np.int32)
    mask32 = np.asarray(neighbor_mask, dtype=np.int32)
    x = np.ascontiguousarray(np.asarray(x, dtype=np.float32))
    xtab = np.ascontiguousarray(x.astype(bf16))
    W1h = np.ascontiguousarray(
        np.asarray(W1, np.float32).transpose(1, 0, 2).reshape(P, K * P).astype(bf16))
    W2h = np.ascontiguousarray(
        np.asarray(W2, np.float32).transpose(1, 0, 2).reshape(P, K * P).astype(bf16))

    # conv2 table position of global row g (h table is chunk-major:
    # ((r//3584)*8 + q)*3584 + r%3584)
    q, r = idx32 // SHARD, idx32 % SHARD
    idx_h = ((r // 3584) * NCORES + q) * 3584 + r % 3584

    vv = np.arange(NPAD).reshape(NT, 4, P)       # local voxel id = 512t + 128j + p
    valid = vv < SHARD
    in_maps = []
    for c in range(NCORES):
        gid = c * SHARD + np.where(valid, vv, 0)
        mb = np.where(valid[..., None], mask32[gid], 0)      # [NT, 4, P, K]
        keep = mb.astype(bool)
        i1 = np.where(keep, idx32[gid], BIG)
        i2 = np.where(keep, idx_h[gid], BIG)
        # layout [NT, P, K*4]: slot m = k*4 + j for voxel 512t + 128j + p
        def lay(a, dt):
            return np.ascontiguousarray(a.transpose(0, 2, 3, 1).reshape(NT, P, K * 4).astype(dt))
        in_maps.append({
            "xtab": xtab, "xres": np.ascontiguousarray(x[c * SHARD:(c + 1) * SHARD]),
            "idx1": lay(i1, np.int32), "idx2": lay(i2, np.int32),
            "W1h": W1h, "W2h": W2h,
            "gamma1": np.asarray(gamma1, np.float32), "beta1": np.asarray(beta1, np.float32),
            "gamma2": np.asarray(gamma2, np.float32), "beta2": np.asarray(beta2, np.float32),
        })
    return in_maps


_NC_CACHE = {}


def kernel(**inputs):
    _install_trace_hook()
    from concourse import bass_utils

    cfg = FULL_CFG
    key = "full"
    if key not in _NC_CACHE:
        _NC_CACHE[key] = build_nc(cfg)
    nc = _NC_CACHE[key]
    in_maps = prepare_in_maps(cfg, **inputs)
    trace = bool(int(os.environ.get("BASS_KERNEL_TRACE", "0")))
    res = bass_utils.run_bass_kernel_spmd(
        nc, in_maps, core_ids=list(range(cfg["n_cores"])), trace=trace)
    out = np.concatenate([res.results[c]["out"] for c in range(cfg["n_cores"])], axis=0)
    if trace:
        kernel.last_exec_time_ns = res.exec_time_ns
    return out
